# revision 42
# baseline (speedup 1.0000x reference)
"""Trainium2 Bass kernel for a 2-layer GCN over 2048 independent 25-node
KNN subgraphs (gnn_message_passing). ~35.4us traced vs 74us for the f32r
baseline.

Design:
  - Data parallel over 8 cores: 256 graphs (52 tiles of 5 graphs packed
    into 128 partitions) per core; weights replicated.
  - bf16 operands everywhere (FWL weight loads at ~26ns need non-fp32
    dtype and exactly 128 stationary columns; 1 cy/col matmuls), f32
    PSUM accumulation. rel err ~8e-3 vs the 2e-2 gate.
  - Layer 1 reassociated as (A @ x) @ W0: mT = X-stationary x AT-moving
    (128 cols) then h1 = mT-stationary x W0-moving (256 cols): 400
    moving cols/tile instead of 528 and half the PSUM->SBUF cast bytes.
  - 4-tile pipeline stages: one PSUM tile, one cast and one relu
    instruction per 4 tiles (Act/DVE fixed cost is 130-260ns each);
    cast and relu alternate between DVE and Act to balance both
    engines at ~800ns/stage.
  - Software-pipelined in-order PE stream: stage s issues mm1(s),
    mm2(s-2), p2(s-5); block W1 transforms and outputs trail their
    upstream DVE copies by 3/2 stages. Otherwise the PE stalls on
    every cast/relu semaphore round trip.
  - p2 center columns accumulate 16 tiles per PSUM bank (4 copies
    total); each block's W1 transform, Wlin matmul and output DMA
    stream inside the loop, so the tail is one tiny DMA.
  - Host packs [x | AT | ATc] into one concatenated row per (node,
    tile): each streamed chunk is ONE dma_start (each costs ~650ns of
    serial sequencer time plus ~900ns completion latency) on the SP
    HWDGE ring in strict consumption order; weights ride the GpSimd
    SWDGE ring in parallel.
  - PE warmup on a zeroed tile: 4 tiles upfront plus 4 interleaved
    after stages 0..3. The HAM activity monitor only unthrottles the
    PE clock (1.2 -> 2.4 GHz) after a fully-busy free-running 3.4us
    window and re-throttles on idle windows, so the warmup must drain
    into the DMA-paced region with no PE gaps.
"""

import sys

import ml_dtypes
import numpy as np

for _p in ("/opt/trn_rl_repo", "/opt/trn_rl_repo/concourse"):
    if _p not in sys.path:
        sys.path.insert(0, _p)

import concourse.bass as bass
import concourse.tile as tile
from concourse import bacc, mybir
from concourse.bass_utils import run_bass_kernel_spmd

NCORES = 8
B = 2048            # graphs
K = 25              # nodes per graph
GPC = B // NCORES   # 256 graphs per core
G = 5               # graphs packed per PE tile
NT = (GPC + G - 1) // G   # 52 tiles per core
SLOTS = NT * G      # 260 graph slots per core
TN = 128            # padded nodes per tile (125 real)
CP = 8              # padded center count per tile (5 real)
F0 = 128            # input features
F1 = 256            # hidden features

NTS = 4             # tiles per pipeline stage
NS = NT // NTS      # 13 stages
PB = 16             # max tiles per p2 PSUM block
BLKS = [16, 16, 16, 4]  # p2 block sizes
BLK_OF = {}
for _b, _n in enumerate(BLKS):
    for _t in range(_n):
        BLK_OF[len(BLK_OF)] = (_b, _t, _t == _n - 1)
BLK_START = [sum(BLKS[:b]) for b in range(len(BLKS))]
CHUNKS = [4, 8, 8, 16, 16]  # tiles per streamed xat DMA chunk
WARMUP_TILES = 4    # upfront PE warmup PSUM tiles (4 matmuls each)
INLINE_WARM = 4     # additional warmup tiles interleaved after stages
                    # 0..3: same total warmup matmuls, but they drain into
                    # the DMA-paced region and plug its PE gaps so the HAM
                    # activity monitor neither misses the unthrottle nor
                    # re-throttles mid-loop
D2 = 2              # software-pipeline depth of the W0 stage
D3 = 5              # software-pipeline depth of the p2 stage
DH3 = 3             # stages between a p2 block copy and its W1 transform
DOUT = 2            # stages between a block's W1 transform and its output
CW = F0 + TN + CP   # concatenated per-tile row: x | at | atc

_f32 = mybir.dt.float32
_bf16 = mybir.dt.bfloat16

_compiled = {}


def _build_nc():
    nc = bacc.Bacc("TRN2", target_bir_lowering=False, debug=False,
                   num_devices=NCORES)

    # per-tile concatenated row [x | at | atc], node-major: partitions =
    # node-within-tile, one contiguous run per partition per tile so each
    # chunk is a single DMA with a single completion semaphore
    xat_d = nc.dram_tensor("xat", [TN, NT, CW], _bf16, kind="ExternalInput")
    w0_d = nc.dram_tensor("w0", [F0, F1], _bf16, kind="ExternalInput")
    w1_d = nc.dram_tensor("w1", [128, 2 * F1], _bf16, kind="ExternalInput")
    wl_d = nc.dram_tensor("wl", [128, 2], _bf16, kind="ExternalInput")
    out_d = nc.dram_tensor("out", [1, SLOTS], _f32, kind="ExternalOutput")

    relu = mybir.ActivationFunctionType.Relu
    copyf = mybir.ActivationFunctionType.Copy

    with tile.TileContext(nc) as tc:
        with (
            tc.tile_pool(name="const", bufs=1) as cpool,
            tc.tile_pool(name="mtp", bufs=3) as mtp,
            tc.tile_pool(name="h1p", bufs=4) as h1p,
            tc.tile_pool(name="outp", bufs=1) as outp,
            tc.tile_pool(name="ps_mt", bufs=2, space=bass.MemorySpace.PSUM) as ps_mt,
            tc.tile_pool(name="ps_h1", bufs=2, space=bass.MemorySpace.PSUM) as ps_h1,
            tc.tile_pool(name="ps_p2", bufs=2, space=bass.MemorySpace.PSUM) as ps_p2,
        ):
            # ---- resident inputs ----
            xat_all = cpool.tile([TN, NT, CW], _bf16)
            w0 = cpool.tile([F0, F1], _bf16)
            w1 = cpool.tile([128, 2 * F1], _bf16)
            wl = cpool.tile([128, 2], _bf16)

            # xat chunks on the SP HWDGE ring in strict consumption order
            # (graduated sizes keep each PE stall under the HAM idle
            # window); weights on the GpSimd SWDGE ring in parallel
            nc.gpsimd.dma_start(w0[:], w0_d[:])
            nc.gpsimd.dma_start(w1[:], w1_d[:])
            nc.gpsimd.dma_start(wl[:], wl_d[:])
            bounds = np.cumsum([0] + CHUNKS)
            for c in range(len(CHUNKS)):
                lo, hi = bounds[c], bounds[c + 1]
                nc.sync.dma_start(xat_all[:, lo:hi, :], xat_d[:, lo:hi, :])

            # ---- PE warmup on a zeroed tile (no DMA deps): keeps the PE
            # busy until real data lands so HAM unthrottles early ----
            warm = cpool.tile([128, F1], _bf16)
            nc.vector.memset(warm[:], 0)
            for _ in range(WARMUP_TILES):
                wp = ps_h1.tile([128, NTS, F1], _f32, name="h1_ps")
                for j in range(NTS):
                    nc.tensor.matmul(wp[:, j, :], warm[:, 0:128], warm[:],
                                     start=True, stop=True)

            # p2 accumulator: [f-chunk, tile, center], bf16 for the final
            # weight-stationary W1 transform
            p2a = cpool.tile([128, 2, NT, G], _bf16)

            h3_sb = cpool.tile([128, 2, SLOTS], _bf16)

            out_sb = outp.tile([1, SLOTS], _f32)

            h3_pss = {}

            def emit_h3(blk, n, s):
                # W1 transform for one copied p2 block (weight stationary);
                # its relu alternates engines by block
                o = BLK_START[blk] * G
                w = n * G
                h3_ps = ps_p2.tile([128, 2, PB * G], _f32, name="p2_ps")
                for fo in range(2):
                    for fi in range(2):
                        nc.tensor.matmul(
                            h3_ps[:, fo, 0:w],
                            w1[:, fi * F1 + fo * 128:fi * F1 + fo * 128 + 128],
                            p2a[:, fi, BLK_START[blk]:BLK_START[blk] + n, :],
                            start=(fi == 0), stop=(fi == 1))
                if blk % 2 == 0:
                    nc.scalar.activation(h3_sb[:, :, o:o + w],
                                         h3_ps[:, :, 0:w], relu)
                else:
                    nc.vector.tensor_scalar_max(h3_sb[:, :, o:o + w],
                                                h3_ps[:, :, 0:w], 0.0)
                h3_pss[blk] = h3_ps

            def emit_out(blk, n, s):
                # out = relu(h3).T @ Wlin for this block; the wl matmuls
                # reuse a spare row of the block's h3 PSUM tile, and the
                # result streams to HBM so the final tail is one tiny DMA
                o = BLK_START[blk] * G
                w = n * G
                h3_ps = h3_pss.pop(blk)
                for fo in range(2):
                    nc.tensor.matmul(h3_ps[0:1, 1, 0:w], wl[:, fo:fo + 1],
                                     h3_sb[:, fo, o:o + w],
                                     start=(fo == 0), stop=(fo == 1))
                nc.vector.tensor_copy(out_sb[0:1, o:o + w],
                                      h3_ps[0:1, 1, 0:w])
                nc.sync.dma_start(out_d[:, o:o + w], out_sb[:, o:o + w])

            mt_sbs = {}
            h1_sbs = {}
            p2_ps = None
            pending_h3 = []
            pending_out = []
            # ---- software-pipelined stage loop ----
            for s in range(NS + D3 + 1):
                # W1 transforms (then output blocks) whose upstream copy
                # has had DH3/DOUT stages to clear the DVE queue (keeps
                # the PE from stalling on cross-engine round trips)
                while pending_out and pending_out[0][0] + DOUT <= s:
                    _, blk, n = pending_out.pop(0)
                    emit_out(blk, n, s)
                while pending_h3 and pending_h3[0][0] + DH3 <= s:
                    _, blk, n = pending_h3.pop(0)
                    emit_h3(blk, n, s)
                    pending_out.append((s, blk, n))
                if s < NS:
                    # mT[f, t] = sum_s x[s, f] * at[s, t]  (= (A @ x).T)
                    mt_ps = ps_mt.tile([128, NTS, TN], _f32)
                    for j in range(NTS):
                        i = s * NTS + j
                        nc.tensor.matmul(mt_ps[:, j, :],
                                         xat_all[:, i, 0:F0],
                                         xat_all[:, i, F0:F0 + TN],
                                         start=True, stop=True)
                    mt_sb = mtp.tile([128, NTS, TN], _bf16)
                    if s % 2 == 0:
                        nc.vector.tensor_copy(mt_sb[:], mt_ps[:])
                    else:
                        nc.scalar.activation(mt_sb[:], mt_ps[:], copyf)
                    mt_sbs[s] = mt_sb

                if D2 <= s < NS + D2:
                    # h1[t, fo] = sum_f mT[f, t] * W0[f, fo]
                    g = s - D2
                    mt_sb = mt_sbs.pop(g)
                    h1_ps = ps_h1.tile([128, NTS, F1], _f32, name="h1_ps")
                    for j in range(NTS):
                        nc.tensor.matmul(h1_ps[:, j, :], mt_sb[:, j, :], w0[:],
                                         start=True, stop=True)
                    h1_sb = h1p.tile([128, NTS, F1], _bf16)
                    if s % 2 == 0:
                        nc.scalar.activation(h1_sb[:], h1_ps[:], relu)
                    else:
                        nc.vector.tensor_scalar_max(h1_sb[:], h1_ps[:], 0.0)
                    h1_sbs[g] = h1_sb

                if D3 <= s < NS + D3:
                    # p2T[f, tc] = sum_s h1[s, f] * ATc[s, tc]
                    q = s - D3
                    h1_sb = h1_sbs.pop(q)
                    for j in range(NTS):
                        i = q * NTS + j
                        blk, u, last = BLK_OF[i]
                        if u == 0:
                            p2_ps = ps_p2.tile([128, 2, PB, CP], _f32,
                                               name="p2_ps")
                        for c in range(2):
                            nc.tensor.matmul(p2_ps[:, c, u, :],
                                             h1_sb[:, j, c * 128:(c + 1) * 128],
                                             xat_all[:, i, F0 + TN:CW],
                                             start=True, stop=True)
                        if last:
                            # copy this block's centers out; its W1
                            # transform is emitted DH3 stages later
                            n = BLKS[blk]
                            b0 = BLK_START[blk]
                            nc.vector.tensor_copy(
                                p2a[:, :, b0:b0 + n, :],
                                p2_ps[:, :, 0:n, 0:G])
                            pending_h3.append((s, blk, n))

                if s < INLINE_WARM:
                    wp = ps_h1.tile([128, NTS, F1], _f32, name="h1_ps")
                    for j in range(NTS):
                        nc.tensor.matmul(wp[:, j, :], warm[:, 0:128], warm[:],
                                         start=True, stop=True)

            while pending_h3 or pending_out:
                while pending_out:
                    _, blk, n = pending_out.pop(0)
                    emit_out(blk, n, blk)
                if pending_h3:
                    _, blk, n = pending_h3.pop(0)
                    emit_h3(blk, n, blk)
                    pending_out.append((0, blk, n))

    nc.compile()
    return nc


def _get_nc():
    if "nc" not in _compiled:
        _compiled["nc"] = _build_nc()
    return _compiled["nc"]


def _host_prep(x, edge_weight, W0, W1, Wlin, edge_index):
    bf16 = ml_dtypes.bfloat16
    src = edge_index[0].astype(np.int64)
    tgt = edge_index[1].astype(np.int64)
    b = src // K
    sl = src - b * K
    tl = tgt - (tgt // K) * K

    # dense raw adjacency per graph, indexed [b, t, s]
    idx = (b * K + tl) * K + sl
    Araw = np.bincount(idx, weights=edge_weight.astype(np.float64),
                       minlength=B * K * K).astype(np.float32).reshape(B, K, K)
    deg = Araw.sum(axis=2)                      # weighted in-degree [B, K]
    with np.errstate(divide="ignore"):
        dinv = np.where(deg > 0, 1.0 / np.sqrt(deg), 0.0).astype(np.float32)
    An = Araw * dinv[:, :, None] * dinv[:, None, :]   # [b, t, s]
    ATn = np.ascontiguousarray(An.transpose(0, 2, 1))  # [b, s, t]

    # scatter graphs into per-core padded slots
    ATs = np.zeros((NCORES, SLOTS, K, K), np.float32)
    ATs[:, :GPC] = ATn.reshape(NCORES, GPC, K, K)
    ATs = ATs.reshape(NCORES, NT, G, K, K)

    # block-diagonal AT per tile, zero padded to 128x128
    at = np.zeros((NCORES, NT, TN, TN), np.float32)
    bd = at[:, :, :G * K, :G * K].reshape(NCORES, NT, G, K, G, K)
    atc = np.zeros((NCORES, NT, TN, CP), np.float32)
    cent = atc[:, :, :G * K, :G].reshape(NCORES, NT, G, K, G)
    for g in range(G):
        bd[:, :, g, :, g, :] = ATs[:, :, g]          # [s, t] block
        cent[:, :, g, :, g] = ATs[:, :, g, :, 0]     # center (t_local=0) col
    # partition-major (node-within-tile first) device layout
    at = np.ascontiguousarray(at.transpose(0, 2, 1, 3)).astype(bf16)
    atc = np.ascontiguousarray(atc.transpose(0, 2, 1, 3)).astype(bf16)

    # x node-major per tile: [core, s, tile, f]
    xp = np.zeros((NCORES, NT * G * K, F0), np.float32)
    xp[:, :GPC * K] = x.reshape(NCORES, GPC * K, F0)
    xq = np.zeros((NCORES, NT, TN, F0), np.float32)
    xq[:, :, :G * K] = xp.reshape(NCORES, NT, G * K, F0)
    xq = np.ascontiguousarray(xq.transpose(0, 2, 1, 3)).astype(bf16)

    # one concatenated [x | at | atc] row per (node, tile)
    xat = np.concatenate([xq, at, atc], axis=3)

    w1 = np.concatenate([W1[0:128, :], W1[128:256, :]], axis=1).astype(bf16)
    wl = np.ascontiguousarray(Wlin.reshape(2, 128).T).astype(bf16)
    w0 = W0.astype(bf16)

    in_maps = []
    for c in range(NCORES):
        in_maps.append({
            "xat": np.ascontiguousarray(xat[c]),
            "w0": w0,
            "w1": np.ascontiguousarray(w1),
            "wl": wl,
        })
    return in_maps


def _run(inputs, trace=False):
    nc = _get_nc()
    in_maps = _host_prep(**inputs)
    res = run_bass_kernel_spmd(nc, in_maps, core_ids=list(range(NCORES)),
                               trace=trace)
    out = np.empty((B, 1), np.float32)
    for c in range(NCORES):
        out[c * GPC:(c + 1) * GPC, 0] = res.results[c]["out"][0, :GPC]
    return out, res


def kernel(**inputs):
    out, _ = _run(inputs, trace=False)
    return out


# revision 45
# speedup vs baseline: 1.0305x; 1.0305x over previous
"""Trainium2 Bass kernel for a 2-layer GCN over 2048 independent 25-node
KNN subgraphs (gnn_message_passing). ~35.4us traced vs 74us for the f32r
baseline.

Design:
  - Data parallel over 8 cores: 256 graphs (52 tiles of 5 graphs packed
    into 128 partitions) per core; weights replicated.
  - bf16 operands everywhere (FWL weight loads at ~26ns need non-fp32
    dtype and exactly 128 stationary columns; 1 cy/col matmuls), f32
    PSUM accumulation. rel err ~8e-3 vs the 2e-2 gate.
  - Layer 1 reassociated as (A @ x) @ W0: mT = X-stationary x AT-moving
    (128 cols) then h1 = mT-stationary x W0-moving (256 cols): 400
    moving cols/tile instead of 528 and half the PSUM->SBUF cast bytes.
  - 4-tile pipeline stages: one PSUM tile, one cast and one relu
    instruction per 4 tiles (Act/DVE fixed cost is 130-260ns each);
    cast and relu alternate between DVE and Act to balance both
    engines at ~800ns/stage.
  - Software-pipelined in-order PE stream: stage s issues mm1(s),
    mm2(s-2), p2(s-5); block W1 transforms and outputs trail their
    upstream DVE copies by 3/2 stages. Otherwise the PE stalls on
    every cast/relu semaphore round trip.
  - p2 center columns accumulate 16 tiles per PSUM bank (4 copies
    total); each block's W1 transform, Wlin matmul and output DMA
    stream inside the loop, so the tail is one tiny DMA.
  - Host packs [x | AT | ATc] into one concatenated row per (node,
    tile): each streamed chunk is ONE dma_start (each costs ~650ns of
    serial sequencer time plus ~900ns completion latency) on the SP
    HWDGE ring in strict consumption order; weights ride the GpSimd
    SWDGE ring in parallel.
  - PE warmup on a zeroed tile: 4 tiles upfront plus 4 interleaved
    after stages 0..3. The HAM activity monitor only unthrottles the
    PE clock (1.2 -> 2.4 GHz) after a fully-busy free-running 3.4us
    window and re-throttles on idle windows, so the warmup must drain
    into the DMA-paced region with no PE gaps.
"""

import sys

import ml_dtypes
import numpy as np

for _p in ("/opt/trn_rl_repo", "/opt/trn_rl_repo/concourse"):
    if _p not in sys.path:
        sys.path.insert(0, _p)

import concourse.bass as bass
import concourse.tile as tile
from concourse import bacc, mybir
from concourse.bass_utils import run_bass_kernel_spmd

NCORES = 8
B = 2048            # graphs
K = 25              # nodes per graph
GPC = B // NCORES   # 256 graphs per core
G = 5               # graphs packed per PE tile
NT = (GPC + G - 1) // G   # 52 tiles per core
SLOTS = NT * G      # 260 graph slots per core
TN = 128            # padded nodes per tile (125 real)
CP = 8              # padded center count per tile (5 real)
F0 = 128            # input features
F1 = 256            # hidden features

NTS = 4             # tiles per pipeline stage
NS = NT // NTS      # 13 stages
PB = 16             # max tiles per p2 PSUM block
BLKS = [16, 16, 16, 4]  # p2 block sizes
BLK_OF = {}
for _b, _n in enumerate(BLKS):
    for _t in range(_n):
        BLK_OF[len(BLK_OF)] = (_b, _t, _t == _n - 1)
BLK_START = [sum(BLKS[:b]) for b in range(len(BLKS))]
CHUNKS = [4, 8, 10, 14, 16]  # tiles per streamed xat DMA chunk
WARMUP_TILES = 4    # upfront PE warmup PSUM tiles (4 matmuls each)
INLINE_WARM = 4     # additional warmup tiles interleaved after stages
                    # 0..3: same total warmup matmuls, but they drain into
                    # the DMA-paced region and plug its PE gaps so the HAM
                    # activity monitor neither misses the unthrottle nor
                    # re-throttles mid-loop
D2 = 2              # software-pipeline depth of the W0 stage
D3 = 5              # software-pipeline depth of the p2 stage
# stages between a p2 block copy and its W1 transform / between the W1
# transform and its output: generous mid-loop (avoid PE stalls on the
# DVE queue), tight for the last blocks so their chains overlap the
# loop drain instead of serializing after it
DH3 = {0: 3, 1: 3, 2: 1, 3: 1}
DOUT = {0: 2, 1: 2, 2: 1, 3: 1}
CW = F0 + TN + CP   # concatenated per-tile row: x | at | atc

_f32 = mybir.dt.float32
_bf16 = mybir.dt.bfloat16

_compiled = {}


def _build_nc():
    nc = bacc.Bacc("TRN2", target_bir_lowering=False, debug=False,
                   num_devices=NCORES)

    # per-tile concatenated row [x | at | atc], node-major: partitions =
    # node-within-tile, one contiguous run per partition per tile so each
    # chunk is a single DMA with a single completion semaphore
    xat_d = nc.dram_tensor("xat", [TN, NT, CW], _bf16, kind="ExternalInput")
    w0_d = nc.dram_tensor("w0", [F0, F1], _bf16, kind="ExternalInput")
    w1_d = nc.dram_tensor("w1", [128, 2 * F1], _bf16, kind="ExternalInput")
    wl_d = nc.dram_tensor("wl", [128, 2], _bf16, kind="ExternalInput")
    out_d = nc.dram_tensor("out", [1, SLOTS], _f32, kind="ExternalOutput")

    relu = mybir.ActivationFunctionType.Relu
    copyf = mybir.ActivationFunctionType.Copy

    with tile.TileContext(nc) as tc:
        with (
            tc.tile_pool(name="const", bufs=1) as cpool,
            tc.tile_pool(name="mtp", bufs=3) as mtp,
            tc.tile_pool(name="h1p", bufs=4) as h1p,
            tc.tile_pool(name="outp", bufs=1) as outp,
            tc.tile_pool(name="ps_mt", bufs=2, space=bass.MemorySpace.PSUM) as ps_mt,
            tc.tile_pool(name="ps_h1", bufs=2, space=bass.MemorySpace.PSUM) as ps_h1,
            tc.tile_pool(name="ps_p2", bufs=2, space=bass.MemorySpace.PSUM) as ps_p2,
        ):
            # ---- resident inputs ----
            xat_all = cpool.tile([TN, NT, CW], _bf16)
            w0 = cpool.tile([F0, F1], _bf16)
            w1 = cpool.tile([128, 2 * F1], _bf16)
            wl = cpool.tile([128, 2], _bf16)

            # xat chunks on the SP HWDGE ring in strict consumption order
            # (graduated sizes keep each PE stall under the HAM idle
            # window); weights on the GpSimd SWDGE ring in parallel
            nc.gpsimd.dma_start(w0[:], w0_d[:])
            nc.gpsimd.dma_start(w1[:], w1_d[:])
            nc.gpsimd.dma_start(wl[:], wl_d[:])
            bounds = np.cumsum([0] + CHUNKS)
            for c in range(len(CHUNKS)):
                lo, hi = bounds[c], bounds[c + 1]
                nc.sync.dma_start(xat_all[:, lo:hi, :], xat_d[:, lo:hi, :])

            # ---- PE warmup on a zeroed tile (no DMA deps): keeps the PE
            # busy until real data lands so HAM unthrottles early ----
            warm = cpool.tile([128, F1], _bf16)
            nc.vector.memset(warm[:], 0)
            for _ in range(WARMUP_TILES):
                wp = ps_h1.tile([128, NTS, F1], _f32, name="h1_ps")
                for j in range(NTS):
                    nc.tensor.matmul(wp[:, j, :], warm[:, 0:128], warm[:],
                                     start=True, stop=True)

            # p2 accumulator: [f-chunk, tile, center], bf16 for the final
            # weight-stationary W1 transform
            p2a = cpool.tile([128, 2, NT, G], _bf16)

            h3_sb = cpool.tile([128, 2, SLOTS], _bf16)

            out_sb = outp.tile([1, SLOTS], _f32)

            h3_pss = {}

            def emit_h3(blk, n, s):
                # W1 transform for one copied p2 block (weight stationary);
                # its relu alternates engines by block
                o = BLK_START[blk] * G
                w = n * G
                h3_ps = ps_p2.tile([128, 2, PB * G], _f32, name="p2_ps")
                for fo in range(2):
                    for fi in range(2):
                        nc.tensor.matmul(
                            h3_ps[:, fo, 0:w],
                            w1[:, fi * F1 + fo * 128:fi * F1 + fo * 128 + 128],
                            p2a[:, fi, BLK_START[blk]:BLK_START[blk] + n, :],
                            start=(fi == 0), stop=(fi == 1))
                if blk % 2 == 0:
                    nc.scalar.activation(h3_sb[:, :, o:o + w],
                                         h3_ps[:, :, 0:w], relu)
                else:
                    nc.vector.tensor_scalar_max(h3_sb[:, :, o:o + w],
                                                h3_ps[:, :, 0:w], 0.0)
                h3_pss[blk] = h3_ps

            def emit_out(blk, n, s):
                # out = relu(h3).T @ Wlin for this block; the wl matmuls
                # reuse a spare row of the block's h3 PSUM tile, and the
                # result streams to HBM so the final tail is one tiny DMA
                o = BLK_START[blk] * G
                w = n * G
                h3_ps = h3_pss.pop(blk)
                for fo in range(2):
                    nc.tensor.matmul(h3_ps[0:1, 1, 0:w], wl[:, fo:fo + 1],
                                     h3_sb[:, fo, o:o + w],
                                     start=(fo == 0), stop=(fo == 1))
                nc.vector.tensor_copy(out_sb[0:1, o:o + w],
                                      h3_ps[0:1, 1, 0:w])
                nc.sync.dma_start(out_d[:, o:o + w], out_sb[:, o:o + w])

            mt_sbs = {}
            h1_sbs = {}
            p2_ps = None
            pending_h3 = []
            pending_out = []
            # ---- software-pipelined stage loop ----
            for s in range(NS + D3 + 1):
                # W1 transforms (then output blocks) whose upstream copy
                # has had DH3/DOUT stages to clear the DVE queue (keeps
                # the PE from stalling on cross-engine round trips)
                while pending_out and pending_out[0][0] + DOUT[pending_out[0][1]] <= s:
                    _, blk, n = pending_out.pop(0)
                    emit_out(blk, n, s)
                while pending_h3 and pending_h3[0][0] + DH3[pending_h3[0][1]] <= s:
                    _, blk, n = pending_h3.pop(0)
                    emit_h3(blk, n, s)
                    pending_out.append((s, blk, n))
                if s < NS:
                    # mT[f, t] = sum_s x[s, f] * at[s, t]  (= (A @ x).T)
                    mt_ps = ps_mt.tile([128, NTS, TN], _f32)
                    for j in range(NTS):
                        i = s * NTS + j
                        nc.tensor.matmul(mt_ps[:, j, :],
                                         xat_all[:, i, 0:F0],
                                         xat_all[:, i, F0:F0 + TN],
                                         start=True, stop=True)
                    mt_sb = mtp.tile([128, NTS, TN], _bf16)
                    if s % 2 == 0:
                        nc.vector.tensor_copy(mt_sb[:], mt_ps[:])
                    else:
                        nc.scalar.activation(mt_sb[:], mt_ps[:], copyf)
                    mt_sbs[s] = mt_sb

                if D2 <= s < NS + D2:
                    # h1[t, fo] = sum_f mT[f, t] * W0[f, fo]
                    g = s - D2
                    mt_sb = mt_sbs.pop(g)
                    h1_ps = ps_h1.tile([128, NTS, F1], _f32, name="h1_ps")
                    for j in range(NTS):
                        nc.tensor.matmul(h1_ps[:, j, :], mt_sb[:, j, :], w0[:],
                                         start=True, stop=True)
                    h1_sb = h1p.tile([128, NTS, F1], _bf16)
                    if s % 2 == 0:
                        nc.scalar.activation(h1_sb[:], h1_ps[:], relu)
                    else:
                        nc.vector.tensor_scalar_max(h1_sb[:], h1_ps[:], 0.0)
                    h1_sbs[g] = h1_sb

                if D3 <= s < NS + D3:
                    # p2T[f, tc] = sum_s h1[s, f] * ATc[s, tc]
                    q = s - D3
                    h1_sb = h1_sbs.pop(q)
                    for j in range(NTS):
                        i = q * NTS + j
                        blk, u, last = BLK_OF[i]
                        if u == 0:
                            p2_ps = ps_p2.tile([128, 2, PB, CP], _f32,
                                               name="p2_ps")
                        for c in range(2):
                            nc.tensor.matmul(p2_ps[:, c, u, :],
                                             h1_sb[:, j, c * 128:(c + 1) * 128],
                                             xat_all[:, i, F0 + TN:CW],
                                             start=True, stop=True)
                        if last:
                            # copy this block's centers out; its W1
                            # transform is emitted DH3 stages later
                            n = BLKS[blk]
                            b0 = BLK_START[blk]
                            nc.vector.tensor_copy(
                                p2a[:, :, b0:b0 + n, :],
                                p2_ps[:, :, 0:n, 0:G])
                            pending_h3.append((s, blk, n))

                if s < INLINE_WARM:
                    wp = ps_h1.tile([128, NTS, F1], _f32, name="h1_ps")
                    for j in range(NTS):
                        nc.tensor.matmul(wp[:, j, :], warm[:, 0:128], warm[:],
                                         start=True, stop=True)

            while pending_h3 or pending_out:
                while pending_out:
                    _, blk, n = pending_out.pop(0)
                    emit_out(blk, n, blk)
                if pending_h3:
                    _, blk, n = pending_h3.pop(0)
                    emit_h3(blk, n, blk)
                    pending_out.append((0, blk, n))

    nc.compile()
    return nc


def _get_nc():
    if "nc" not in _compiled:
        _compiled["nc"] = _build_nc()
    return _compiled["nc"]


def _host_prep(x, edge_weight, W0, W1, Wlin, edge_index):
    bf16 = ml_dtypes.bfloat16
    src = edge_index[0].astype(np.int64)
    tgt = edge_index[1].astype(np.int64)
    b = src // K
    sl = src - b * K
    tl = tgt - (tgt // K) * K

    # dense raw adjacency per graph, indexed [b, t, s]
    idx = (b * K + tl) * K + sl
    Araw = np.bincount(idx, weights=edge_weight.astype(np.float64),
                       minlength=B * K * K).astype(np.float32).reshape(B, K, K)
    deg = Araw.sum(axis=2)                      # weighted in-degree [B, K]
    with np.errstate(divide="ignore"):
        dinv = np.where(deg > 0, 1.0 / np.sqrt(deg), 0.0).astype(np.float32)
    An = Araw * dinv[:, :, None] * dinv[:, None, :]   # [b, t, s]
    ATn = np.ascontiguousarray(An.transpose(0, 2, 1))  # [b, s, t]

    # scatter graphs into per-core padded slots
    ATs = np.zeros((NCORES, SLOTS, K, K), np.float32)
    ATs[:, :GPC] = ATn.reshape(NCORES, GPC, K, K)
    ATs = ATs.reshape(NCORES, NT, G, K, K)

    # block-diagonal AT per tile, zero padded to 128x128
    at = np.zeros((NCORES, NT, TN, TN), np.float32)
    bd = at[:, :, :G * K, :G * K].reshape(NCORES, NT, G, K, G, K)
    atc = np.zeros((NCORES, NT, TN, CP), np.float32)
    cent = atc[:, :, :G * K, :G].reshape(NCORES, NT, G, K, G)
    for g in range(G):
        bd[:, :, g, :, g, :] = ATs[:, :, g]          # [s, t] block
        cent[:, :, g, :, g] = ATs[:, :, g, :, 0]     # center (t_local=0) col
    # partition-major (node-within-tile first) device layout
    at = np.ascontiguousarray(at.transpose(0, 2, 1, 3)).astype(bf16)
    atc = np.ascontiguousarray(atc.transpose(0, 2, 1, 3)).astype(bf16)

    # x node-major per tile: [core, s, tile, f]
    xp = np.zeros((NCORES, NT * G * K, F0), np.float32)
    xp[:, :GPC * K] = x.reshape(NCORES, GPC * K, F0)
    xq = np.zeros((NCORES, NT, TN, F0), np.float32)
    xq[:, :, :G * K] = xp.reshape(NCORES, NT, G * K, F0)
    xq = np.ascontiguousarray(xq.transpose(0, 2, 1, 3)).astype(bf16)

    # one concatenated [x | at | atc] row per (node, tile)
    xat = np.concatenate([xq, at, atc], axis=3)

    w1 = np.concatenate([W1[0:128, :], W1[128:256, :]], axis=1).astype(bf16)
    wl = np.ascontiguousarray(Wlin.reshape(2, 128).T).astype(bf16)
    w0 = W0.astype(bf16)

    in_maps = []
    for c in range(NCORES):
        in_maps.append({
            "xat": np.ascontiguousarray(xat[c]),
            "w0": w0,
            "w1": np.ascontiguousarray(w1),
            "wl": wl,
        })
    return in_maps


def _run(inputs, trace=False):
    nc = _get_nc()
    in_maps = _host_prep(**inputs)
    res = run_bass_kernel_spmd(nc, in_maps, core_ids=list(range(NCORES)),
                               trace=trace)
    out = np.empty((B, 1), np.float32)
    for c in range(NCORES):
        out[c * GPC:(c + 1) * GPC, 0] = res.results[c]["out"][0, :GPC]
    return out, res


def kernel(**inputs):
    out, _ = _run(inputs, trace=False)
    return out
